# revision 50
# baseline (speedup 1.0000x reference)
"""BiAttention kernel for Trainium2, 8 NeuronCores, data-parallel over batch.

Reference computation (per batch b):
    S[i,j] = w1.c_i + w2.q_j + w3.(c_i*q_j)
    A      = softmax(S, axis=j)
    U[i]   = sum_j A[i,j] q_j
    bmax_i = max_j A[i,j]
    h      = sum_i bmax_i c_i
    G      = concat([c, U, c*U, c*h], axis=-1)

Key restructuring (vs the row-major baseline):
  - softmax over j is invariant to the w1.c_i term -> w1 is dead.
  - S is computed TRANSPOSED: S^T[j,i] = qT (x) (w3*c)^T via PE, with
    i blocked 512 wide.  exp(S^T + s_q[j]) is ONE activation per psum
    bank with s_q as a per-partition bias -> A^T lands in SBUF already
    in the layout the U matmul needs as lhsT.  This deletes the 8
    PE-transposes of A per row-tile AND their ACT evacuations that
    dominated the baseline.
  - Z_i falls out of the U matmul via a ones-column appended to q.
  - bmax_i: DVE 3-op max tree over the 8 j-chunks, then 4 small PE
    transposes + one DVE free-axis max reduce.
  - h = b^T c accumulated per-group on PE; broadcast of h to 128
    partitions via a rank-1 ones (x) h matmul (no DRAM round-trip).
  - A, q are bf16 on the U path (PE full rate at any N, 2x DVE).
    S matmul runs in float32r.
  - Software-pipelined over 8 groups of 4 row-tiles so PE never waits
    on ACT/DVE evacuations: PE stream is U(k), T(k+2), MT(k), S(k+1),
    H(k) per iteration.
  - Engine placement tuned to measured HW: all elementwise on DVE
    (GpSimd shares its SBUF ports at ~half the rate), the c->fp32r
    rounding casts on ACT, all output DMAs issued from SP (idle after
    the prologue), inputs as 6 DMAs ordered by first use.
"""

import sys

if "/opt/trn_rl_repo" not in sys.path:
    sys.path.insert(0, "/opt/trn_rl_repo")

from contextlib import ExitStack

import numpy as np

import concourse.bass as bass
import concourse.bacc as bacc_mod
import concourse.tile as tile
from concourse import mybir
from concourse.bass_utils import run_bass_kernel_spmd
from concourse.masks import make_identity

B, Tc, Tq, D = 8, 4096, 1024, 256
P = 128
NT = Tc // P  # 32 context row-tiles
GT = 4  # row-tiles per group
NG = NT // GT  # 8 groups
GW = GT * P  # 512 rows per group
JC = Tq // P  # 8 question chunks
KC = D // P  # 2 feature chunks
N_CORES = 8
F32 = mybir.dt.float32
R32 = mybir.dt.float32r
BF16 = mybir.dt.bfloat16
EXP = mybir.ActivationFunctionType.Exp
COPY = mybir.ActivationFunctionType.Copy
ADT = BF16  # dtype of A / q_aug on the U path


def _build_program() -> bass.Bass:
    nc = bacc_mod.Bacc()
    c_dram = nc.declare_dram_parameter("context", [Tc, D], F32, isOutput=False)
    q_dram = nc.declare_dram_parameter("question", [Tq, D], F32, isOutput=False)
    w_dram = nc.declare_dram_parameter("w", [3 * D, 1], F32, isOutput=False)
    g_dram = nc.declare_dram_parameter("out", [Tc, 4 * D], F32, isOutput=True)

    with ExitStack() as ctx:
        tc = ctx.enter_context(tile.TileContext(nc))
        singles = ctx.enter_context(tc.tile_pool(name="singles", bufs=1))
        work = ctx.enter_context(tc.tile_pool(name="work", bufs=2))
        ps_s = ctx.enter_context(tc.tile_pool(name="ps_s", bufs=4, space="PSUM"))
        ps_tp = ctx.enter_context(tc.tile_pool(name="ps_tp", bufs=2, space="PSUM"))
        ps_u = ctx.enter_context(tc.tile_pool(name="ps_u", bufs=2, space="PSUM"))

        # ---------------- prologue ----------------
        # consolidated input DMAs, ordered by when consumers need them:
        # first c groups 0-1, then q, then w (tiny), then the c rest
        c_all = singles.tile([P, NT, D], F32)
        c_allr = singles.tile([P, NT, D], R32)
        q_raw = singles.tile([P, JC, D], F32)
        w_cols = singles.tile([P, 6], F32)
        # c kept twice: exact f32 (outputs, elementwise muls) and an
        # fp32r-rounded copy (matmul operands -- the BIR verifier requires
        # fp32r matmul inputs to come through a rounding op)
        nc.sync.dma_start(
            out=q_raw[:, 0 : JC // 2, :],
            in_=q_dram[0 : Tq // 2, :].rearrange("(jc p) d -> p jc d", p=P),
        )
        nc.sync.dma_start(
            out=q_raw[:, JC // 2 :, :],
            in_=q_dram[Tq // 2 :, :].rearrange("(jc p) d -> p jc d", p=P),
        )
        nc.sync.dma_start(
            out=c_all[:, 0:GT, :],
            in_=c_dram[0:GW, :].rearrange("(g p) d -> p g d", p=P),
        )
        nc.sync.dma_start(
            out=w_cols, in_=w_dram[:].rearrange("(g p) one -> p (g one)", p=P)
        )
        nc.sync.dma_start(
            out=c_all[:, GT : 2 * GT, :],
            in_=c_dram[GW : 2 * GW, :].rearrange("(g p) d -> p g d", p=P),
        )
        nc.sync.dma_start(
            out=c_all[:, 2 * GT : 3 * GT, :],
            in_=c_dram[2 * GW : 3 * GW, :].rearrange("(g p) d -> p g d", p=P),
        )
        nc.sync.dma_start(
            out=c_all[:, 3 * GT :, :],
            in_=c_dram[3 * GW :, :].rearrange("(g p) d -> p g d", p=P),
        )
        w3sc = w_cols[:, 4:6]
        w2r = singles.tile([P, KC], R32)
        nc.vector.tensor_copy(w2r, w_cols[:, 2:4])

        def stage_CR(k):
            # on ACT: DVE and GpSimd share SBUF ports and are the busier pool
            nc.scalar.copy(
                c_allr[:, k * GT : (k + 1) * GT, :],
                c_all[:, k * GT : (k + 1) * GT, :],
            )

        ident = singles.tile([P, P], F32)
        make_identity(nc, ident)
        identr = singles.tile([P, P], R32)
        nc.vector.tensor_copy(identr, ident)
        identb = singles.tile([P, P], BF16)
        nc.vector.tensor_copy(identb, ident)

        # q augmented with a ones column (Z) in bf16 for the U matmul
        q_aug = singles.tile([P, JC, D + 4], ADT)
        nc.vector.memset(q_aug[:, :, D : D + 4], 0.0)
        nc.vector.memset(q_aug[:, :, D : D + 1], 1.0)
        for jg in range(2):
            nc.vector.tensor_copy(
                q_aug[:, jg * 4 : (jg + 1) * 4, 0:D], q_raw[:, jg * 4 : (jg + 1) * 4, :]
            )

        # q^T (unscaled, fp32r): lhsT of the S^T matmul + s_q matvec input
        qTr = [singles.tile([P, Tq], R32, name=f"qTr{k}") for k in range(KC)]

        def q_transposes(jg):
            for kc in range(KC):
                tp = ps_tp.tile([P, 512], F32, tag="tp")
                for j4 in range(4):
                    jc = jg * 4 + j4
                    nc.tensor.transpose(
                        tp[:, j4 * P : (j4 + 1) * P],
                        q_raw[:, jc, kc * P : (kc + 1) * P],
                        ident,
                    )
                nc.vector.tensor_copy(qTr[kc][:, jg * 512 : (jg + 1) * 512], tp)

        q_transposes(0)  # q half 0 arrives first; T(0) fills the q1 gap

        # s_q = q @ w2 as a [1, Tq] row (fp32r matmuls need even N), then
        # transposed into per-partition columns for the exp bias
        sq_row = singles.tile([1, Tq], F32)
        sq_col = singles.tile([P, JC], F32)

        def sq_chain():
            for nb in range(2):
                sq_ps = ps_u.tile([1, 512], F32, tag="u")
                for kc in range(KC):
                    nc.tensor.matmul(
                        sq_ps,
                        lhsT=w2r[:, kc : kc + 1],
                        rhs=qTr[kc][:, nb * 512 : (nb + 1) * 512],
                        start=(kc == 0),
                        stop=(kc == KC - 1),
                    )
                nc.vector.tensor_copy(sq_row[:, nb * 512 : (nb + 1) * 512], sq_ps)
            sqt_ps = ps_tp.tile([P, JC], F32, tag="tp")
            for jc in range(JC):
                nc.tensor.transpose(
                    sqt_ps[:, jc : jc + 1],
                    sq_row[:, jc * P : (jc + 1) * P],
                    ident[0:1, 0:1],
                )
            nc.vector.tensor_copy(sq_col, sqt_ps)

        # per-row stats gathered across the loop
        rz_all = singles.tile([P, NT], F32)
        bz_all = singles.tile([P, NT], F32)
        b_all = singles.tile([P, NT], R32)
        h_sb = singles.tile([1, D], F32)
        nc.vector.memset(h_sb, 0.0)
        onesr = singles.tile([1, P], R32)
        nc.vector.memset(onesr.bitcast(F32), 1.0)

        # ---------------- pipelined main loop ----------------
        def stage_T(k):
            # c^T for group k: 8 transposes -> 2 psum banks -> w3-scaled evac
            ctw = work.tile([P, KC, GW], R32, tag="ctw")
            for kc in range(KC):
                tp = ps_tp.tile([P, GW], R32, tag="tp")
                for it in range(GT):
                    t = k * GT + it
                    nc.tensor.transpose(
                        tp[:, it * P : (it + 1) * P],
                        c_allr[:, t, kc * P : (kc + 1) * P],
                        identr,
                    )
                nc.vector.tensor_scalar_mul(
                    ctw[:, kc, :], tp, w3sc[:, kc : kc + 1]
                )
            return ctw

        def stage_S(k, ctw):
            # S^T[j, i] psum banks (one per jc) + exp(.+s_q[j]) -> A^T bf16
            A = work.tile([P, JC, GW], ADT, tag="A")
            for jc in range(JC):
                s_ps = ps_s.tile([P, GW], F32, tag="s")
                for kc in range(KC):
                    nc.tensor.matmul(
                        s_ps,
                        lhsT=qTr[kc][:, jc * P : (jc + 1) * P],
                        rhs=ctw[:, kc, :],
                        start=(kc == 0),
                        stop=(kc == KC - 1),
                    )
                nc.scalar.activation(
                    A[:, jc, :], s_ps, EXP, bias=sq_col[:, jc : jc + 1]
                )
            return A

        def stage_U(k, A, st):
            # U~ = A_raw^T.T @ [q|1]; per row-tile evac U/Z and c*U.
            # The DVE max-tree over A is interleaved with the reciprocals
            # so neither blocks the other's psum/sbuf rotation.
            mx4 = work.tile([P, 4, GW], ADT, tag="mx4", bufs=1)
            mx2 = work.tile([P, 2, GW], ADT, tag="mx2", bufs=1)
            macc = work.tile([P, GW], ADT, tag="macc")
            mx_ops = [
                lambda: nc.vector.tensor_max(mx4, A[:, 0:4, :], A[:, 4:8, :]),
                lambda: nc.vector.tensor_max(mx2, mx4[:, 0:2, :], mx4[:, 2:4, :]),
                lambda: nc.vector.tensor_max(macc, mx2[:, 0, :], mx2[:, 1, :]),
            ]
            for it in range(GT):
                t = k * GT + it
                u_ps = ps_u.tile([P, D + 4], F32, tag="u")
                for jc in range(JC):
                    nc.tensor.matmul(
                        u_ps[:, 0 : D + 1],
                        lhsT=A[:, jc, it * P : (it + 1) * P],
                        rhs=q_aug[:, jc, 0 : D + 1],
                        start=(jc == 0),
                        stop=(jc == JC - 1),
                    )
                if it < 3:
                    mx_ops[it]()
                nc.vector.reciprocal(rz_all[:, t : t + 1], u_ps[:, D : D + 1])
                if k >= NG - 2:
                    # final iterations: scale on DVE so ACT's queue reaches
                    # the last exps sooner (the loop-drain critical path)
                    nc.vector.tensor_scalar_mul(
                        st[:, it, 0:D], u_ps[:, 0:D], rz_all[:, t : t + 1]
                    )
                else:
                    nc.scalar.activation(
                        st[:, it, 0:D], u_ps[:, 0:D], COPY,
                        scale=rz_all[:, t : t + 1],
                    )
                # on DVE, not GpSimd: they share SBUF ports and GpSimd is
                # ~2x slower per element, so it wastes shared port time
                nc.vector.tensor_mul(
                    st[:, it, D : 2 * D], c_all[:, t, :], st[:, it, 0:D]
                )
            return macc

        def stage_MT(k, macc):
            # partition-axis max: transpose macc then free-axis reduce
            mt = ps_tp.tile([P, GT, P], ADT, tag="tp")
            for it in range(GT):
                nc.tensor.transpose(
                    mt[:, it, :], macc[:, it * P : (it + 1) * P], identb
                )
            nc.vector.tensor_reduce(
                out=bz_all[:, k * GT : (k + 1) * GT],
                in_=mt,
                axis=mybir.AxisListType.X,
                op=mybir.AluOpType.max,
            )
            nc.vector.tensor_mul(
                b_all[:, k * GT : (k + 1) * GT],
                bz_all[:, k * GT : (k + 1) * GT],
                rz_all[:, k * GT : (k + 1) * GT],
            )

        def stage_H(k):
            # h partial for group k: b^T c -> psum [1, D] -> h_sb += .
            ph = ps_u.tile([1, D], F32, tag="u")
            for it in range(GT):
                t = k * GT + it
                nc.tensor.matmul(
                    ph,
                    lhsT=b_all[:, t : t + 1],
                    rhs=c_allr[:, t, :],
                    start=(it == 0),
                    stop=(it == GT - 1),
                )
            nc.vector.tensor_add(h_sb, h_sb, ph)

        def stage_OUT(k, st):
            # issue on SP, which is idle after the prologue
            g_rows = g_dram[k * GW : (k + 1) * GW, :]
            nc.sync.dma_start(
                out=g_rows[:, 0:D].rearrange("(g p) d -> p g d", p=P),
                in_=c_all[:, k * GT : (k + 1) * GT, :],
            )
            nc.sync.dma_start(
                out=g_rows[:, D : 3 * D].rearrange("(g p) d -> p g d", p=P),
                in_=st,
            )

        q_transposes(1)
        stage_CR(0)
        ctw = {0: stage_T(0)}
        sq_chain()
        A = {0: stage_S(0, ctw.pop(0))}
        stage_CR(1)
        ctw[1] = stage_T(1)
        for k in range(NG):
            st = work.tile([P, GT, 2 * D], F32, tag="st")
            macc = stage_U(k, A.pop(k), st)
            if k + 2 < NG:
                stage_CR(k + 2)
                ctw[k + 2] = stage_T(k + 2)
            stage_MT(k, macc)
            if k + 1 < NG:
                A[k + 1] = stage_S(k + 1, ctw.pop(k + 1))
            stage_H(k)
            stage_OUT(k, st)

        # ---------------- epilogue: h, then c*h ----------------
        h_row = singles.tile([1, D], R32)
        nc.vector.tensor_copy(h_row, h_sb)
        hb_ps = ps_tp.tile([P, D], F32, tag="tp")
        nc.tensor.matmul(hb_ps, lhsT=onesr, rhs=h_row, start=True, stop=True)
        # replicate h 4x along free so each group is ONE wide DVE mul
        hb4 = singles.tile([P, GT, D], F32)
        nc.vector.tensor_copy(hb4[:, 0, :], hb_ps)
        nc.vector.tensor_copy(hb4[:, 1, :], hb4[:, 0, :])
        nc.vector.tensor_copy(hb4[:, 2:4, :], hb4[:, 0:2, :])

        for k in range(NG):
            ch = work.tile([P, GT, D], F32, tag="ch", bufs=6)
            nc.vector.tensor_mul(ch, c_all[:, k * GT : (k + 1) * GT, :], hb4)
            nc.sync.dma_start(
                out=g_dram[k * GW : (k + 1) * GW, 3 * D : 4 * D].rearrange(
                    "(g p) d -> p g d", p=P
                ),
                in_=ch,
            )

    nc.finalize()
    return nc


_NC_CACHE = None


def kernel(context, question, w):
    global _NC_CACHE
    context = np.asarray(context, dtype=np.float32)
    question = np.asarray(question, dtype=np.float32)
    w = np.asarray(w, dtype=np.float32)

    if _NC_CACHE is None:
        _NC_CACHE = _build_program()
    nc = _NC_CACHE

    in_maps = [
        {"context": context[b], "question": question[b], "w": w} for b in range(B)
    ]
    res = run_bass_kernel_spmd(nc, in_maps, list(range(N_CORES)))
    return np.stack([res.results[b]["out"] for b in range(B)], axis=0)
